# revision 1
# baseline (speedup 1.0000x reference)
"""Trainium2 Bass kernel: MultiHeadLatentAttention.

Problem (hardcoded): B=4, S=1024, HID=2048, NH=16 heads of HD=128, LAT=512,
fp32, causal attention with RoPE, latent-compressed K/V (MLA).

Sharding over 8 NeuronCores: core c = (batch b = c//2, head-group hg = c%2).
Each core handles one batch element and 8 heads (local width HL=1024).

Device-side layout strategy (everything transposed so the contraction dim
always sits on SBUF partitions):
  xT      [HID, S]   (host passes x[b].T)
  QT = (x Wq + bq).T          -> [HL, S]  per head h: QT[h*128:(h+1)*128] = q_h.T
  latT = (x Wdown).T          -> [LAT, S]
  KT = (lat Wk_up).T          -> [HL, S]
  V  = lat Wv_up  (natural)   -> [S, HL]
  RoPE on QT/KT: q*cos + rotate_half(q)*sin, computed as
      qT*cosT + shift64(qT)*sinTe   (sign of sin folded into sinTe by host)
  scoresT_h = k_h @ q_h.T     -> [k, q] blocks   (lhsT = KT block, rhs = QT)
  expT = exp(scoresT/sqrt(128)); diagonal blocks are column-sliced to the
      unmasked range and the residual triangle is zeroed with a binary mask
  sums[1, q]  = ones.T @ expT  (PE reduction over k partitions)
  ctxT_h[d,q] = v_h.T @ ... accumulated:  matmul(lhsT=V block, rhs=expT)
  normalize: bcast = ones_col.T @ sums (K=1 matmul partition-broadcast),
      ctxT *= 1/bcast
  outT_partial = Wo_s.T @ ctxT -> [HID, S], DMA'd from PSUM to DRAM.

Host gathers: out[b] = (outT[2b] + outT[2b+1]).T + bo.
"""

import os

if "axon" not in os.environ.get("JAX_PLATFORMS", ""):
    os.environ["JAX_PLATFORMS"] = "axon"

import numpy as np

import concourse.bacc as bacc
import concourse.mybir as mybir
import concourse.tile as tile
from concourse.bass_utils import run_bass_kernel_spmd

# ---- problem dims (hardcoded per contest rules)
B, S, HID, NH, LAT = 4, 1024, 2048, 16, 512
HD = 128
NHL = NH // 2          # heads per core = 8
HL = NHL * HD          # local head width = 1024
P = 128
KT_H = HID // P        # 16
KT_L = LAT // P        # 4
QCW = 512              # q-chunk width (fp32 matmul moving limit / PSUM bank)
NQC = S // QCW         # 2
SC_SCALE = float(1.0 / np.sqrt(HD))

F32 = mybir.dt.float32
F32R = mybir.dt.float32r

N_CORES = 8


def _rope(nc, pool, raw, out_ap, cosT_sb, sinTe_sb, dve_sin=False,
          dma_shift=False):
    """out = raw * cosT + shift64(raw) * sinTe on a full [128, S] tile."""
    sh = pool.tile([P, S], F32, tag="shift", name="sh")
    if dma_shift:  # use DMA when the HBM queues are idle in this phase
        nc.sync.dma_start(sh[0:64, :], raw[64:128, :])
        nc.sync.dma_start(sh[64:128, :], raw[0:64, :])
    else:
        nc.gpsimd.tensor_copy(sh[0:64, :], raw[64:128, :])
        nc.gpsimd.tensor_copy(sh[64:128, :], raw[0:64, :])
    nc.vector.tensor_mul(out_ap, raw, cosT_sb)
    if dve_sin:
        nc.vector.tensor_mul(sh, sh, sinTe_sb)
    else:
        nc.gpsimd.tensor_mul(sh, sh, sinTe_sb)
    nc.vector.tensor_add(out_ap, out_ap, sh)


def build_bass(loop_iters=None):
    nc = bacc.Bacc("TRN2", target_bir_lowering=False, debug=False, num_devices=8)

    xT = nc.dram_tensor("xT", [HID, S], F32, kind="ExternalInput")[:]
    wq = nc.dram_tensor("wq", [HID, HL], F32, kind="ExternalInput")[:]
    wdown = nc.dram_tensor("wdown", [HID, LAT], F32, kind="ExternalInput")[:]
    wkup = nc.dram_tensor("wkup", [LAT, HL], F32, kind="ExternalInput")[:]
    wvup = nc.dram_tensor("wvup", [LAT, HL], F32, kind="ExternalInput")[:]
    wo = nc.dram_tensor("wo", [HL, HID], F32, kind="ExternalInput")[:]
    bqd = nc.dram_tensor("bq", [P, NHL], F32, kind="ExternalInput")[:]
    cosTd = nc.dram_tensor("cosT", [P, S], F32, kind="ExternalInput")[:]
    sinTed = nc.dram_tensor("sinTe", [P, S], F32, kind="ExternalInput")[:]
    maskTd = nc.dram_tensor("maskT", [P, 3 * P], F32, kind="ExternalInput")[:]
    onescd = nc.dram_tensor("ones_c", [1, P], F32, kind="ExternalInput")[:]
    oneskd = nc.dram_tensor("ones_r", [P, 1], F32, kind="ExternalInput")[:]
    outT = nc.dram_tensor("outT", [HID, S], F32, kind="ExternalOutput")[:]

    import contextlib

    with tile.TileContext(nc) as tc, contextlib.ExitStack() as _les:
        if loop_iters is not None:
            _les.enter_context(tc.For_i(0, loop_iters, 1))
        with (
            tc.tile_pool(name="consts", bufs=1) as consts,
            tc.tile_pool(name="resident", bufs=1) as resident,
            tc.tile_pool(name="psc", bufs=2, space="PSUM") as psc,
        ):
            cosT_sb = consts.tile([P, S], F32)
            sinTe_sb = consts.tile([P, S], F32)
            mask_sb = consts.tile([P, 3 * P], F32)
            bq_sb = consts.tile([P, NHL], F32)
            ones_col = consts.tile([1, P], F32R)
            ones_k = consts.tile([P, 1], F32R)

            latT_sb = resident.tile([P, KT_L, S], F32R)
            qT_sb = resident.tile([P, NHL, S], F32R)
            kT_sb = resident.tile([P, NHL, S], F32R)

            # PSUM accumulators for all projection phases (6 banks; psc has 2)
            pacc_cm = tc.tile_pool(name="pacc", bufs=6, space="PSUM")
            pacc = pacc_cm.__enter__()

            # ---------- phases 1-3: QT (bias+rope), latT, KT (rope)
            with (
                tc.tile_pool(name="xp", bufs=1) as xp,
                tc.tile_pool(name="ws1", bufs=6) as ws1,
                tc.tile_pool(name="rope1", bufs=3) as rp1,
            ):
                xT_sb = xp.tile([P, KT_H, S], F32R)
                nc.sync.dma_start(bq_sb, bqd)

                def proj_og(w_dram, rhs_sb, n_kt, og, wtag, load_x=False,
                            n_oi=2, wpool=None):
                    """One out group: accumulate n_oi x NQC psum tiles."""
                    wpool = wpool or ws1
                    ps = {}
                    for oi in range(n_oi):
                        for ntc in range(NQC):
                            ps[(oi, ntc)] = pacc.tile(
                                [P, QCW], F32, tag="acc", name="acc"
                            )
                    for kt in range(n_kt):
                        if load_x:
                            # stream xT chunk just-in-time (og 0 only)
                            nc.sync.dma_start(
                                xT_sb[:, kt, :],
                                xT[kt * P:(kt + 1) * P, :].bitcast(F32R),
                            )
                        wt = wpool.tile([P, n_oi * P], F32R, tag=wtag, name="wt")
                        nc.sync.dma_start(
                            wt,
                            w_dram[kt * P:(kt + 1) * P,
                                   og * n_oi * P:(og + 1) * n_oi * P
                                   ].bitcast(F32R),
                        )
                        for oi in range(n_oi):
                            for ntc in range(NQC):
                                nc.tensor.matmul(
                                    ps[(oi, ntc)],
                                    lhsT=wt[:, oi * P:(oi + 1) * P],
                                    rhs=rhs_sb[:, kt, ntc * QCW:(ntc + 1) * QCW],
                                    start=(kt == 0),
                                    stop=(kt == n_kt - 1),
                                )
                    return ps

                # QT: bias + rope per head. The first group covers 4 heads
                # (8 accumulators: 6 from pacc + 2 borrowed from the idle
                # attention scores pool) so the PE stays fed for the whole
                # ~29us xT-streaming window.
                def qt_rope(h, ps_oi_ntc):
                    raw = rp1.tile([P, S], F32, tag="raw", name="raw")
                    for ntc in range(NQC):
                        nc.scalar.add(
                            raw[:, ntc * QCW:(ntc + 1) * QCW],
                            ps_oi_ntc[ntc],
                            bq_sb[:, h:h + 1],
                        )
                    _rope(nc, rp1, raw, qT_sb[:, h, :], cosT_sb, sinTe_sb,
                          dve_sin=(h % 2 == 0))

                # mega-group: heads 0-3
                ps = {}
                for oi in range(4):
                    for ntc in range(NQC):
                        pool = pacc if (oi, ntc) < (3, 0) else psc
                        tag = "acc" if pool is pacc else "sc"
                        ps[(oi, ntc)] = pool.tile(
                            [P, QCW], F32, tag=tag, name="acc"
                        )
                for kt in range(KT_H):
                    nc.sync.dma_start(
                        xT_sb[:, kt, :],
                        xT[kt * P:(kt + 1) * P, :].bitcast(F32R),
                    )
                    wt = ws1.tile([P, 4 * P], F32R, tag="wq4", name="wt")
                    nc.sync.dma_start(
                        wt, wq[kt * P:(kt + 1) * P, 0:4 * P].bitcast(F32R)
                    )
                    for oi in range(4):
                        for ntc in range(NQC):
                            nc.tensor.matmul(
                                ps[(oi, ntc)],
                                lhsT=wt[:, oi * P:(oi + 1) * P],
                                rhs=xT_sb[:, kt, ntc * QCW:(ntc + 1) * QCW],
                                start=(kt == 0),
                                stop=(kt == KT_H - 1),
                            )
                nc.sync.dma_start(cosT_sb, cosTd)
                nc.sync.dma_start(sinTe_sb, sinTed)
                for oi in range(4):
                    qt_rope(oi, {ntc: ps[(oi, ntc)] for ntc in range(NQC)})

                # heads 4-7 in pair groups
                for og in range(2, 4):
                    ps = proj_og(wq, xT_sb, KT_H, og, "wq")
                    for oi in range(2):
                        h = og * 2 + oi
                        qt_rope(h, {ntc: ps[(oi, ntc)] for ntc in range(NQC)})

                # latT (og 0 borrows the idle scores-PSUM banks so it can
                # start before QT og3's accumulators drain)
                for og in range(2):
                    if og == 0:
                        ps = {}
                        for oi in range(2):
                            for ntc in range(NQC):
                                pool = pacc if oi == 0 else psc
                                tag = "acc" if pool is pacc else "sc"
                                ps[(oi, ntc)] = pool.tile(
                                    [P, QCW], F32, tag=tag, name="acc"
                                )
                        for kt in range(KT_H):
                            wt = ws1.tile([P, 2 * P], F32R, tag="wd", name="wt")
                            nc.sync.dma_start(
                                wt,
                                wdown[kt * P:(kt + 1) * P, 0:2 * P].bitcast(F32R),
                            )
                            for oi in range(2):
                                for ntc in range(NQC):
                                    nc.tensor.matmul(
                                        ps[(oi, ntc)],
                                        lhsT=wt[:, oi * P:(oi + 1) * P],
                                        rhs=xT_sb[:, kt,
                                                  ntc * QCW:(ntc + 1) * QCW],
                                        start=(kt == 0),
                                        stop=(kt == KT_H - 1),
                                    )
                    else:
                        ps = proj_og(wdown, xT_sb, KT_H, og, "wd")
                    for oi in range(2):
                        for ntc in range(NQC):
                            nc.scalar.copy(
                                latT_sb[:, og * 2 + oi, ntc * QCW:(ntc + 1) * QCW],
                                ps[(oi, ntc)],
                            )

            # xT / ws1 / rope1 freed here

            # ---------- phase 4: V natural [S, HL] (reuses xT's SBUF zone)
            vpool_cm = tc.tile_pool(name="vres", bufs=1)
            vpool = vpool_cm.__enter__()
            v_sb = vpool.tile([P, NHL, HL], F32R)  # [s%128, s//128, hl]
            with tc.tile_pool(name="ws2", bufs=6) as ws2:
                for hlc in range(2):
                    for sg in range(2):
                        ps = {}
                        for si in range(4):
                            ps[si] = pacc.tile([P, QCW], F32, tag="acc", name="acc")
                        for kt in range(KT_L):
                            wt = ws2.tile([P, QCW], F32R, tag="wv", name="wt")
                            nc.sync.dma_start(
                                wt,
                                wvup[kt * P:(kt + 1) * P,
                                     hlc * QCW:(hlc + 1) * QCW].bitcast(F32R),
                            )
                            for si in range(4):
                                st = sg * 4 + si
                                nc.tensor.matmul(
                                    ps[si],
                                    lhsT=latT_sb[:, kt, st * P:(st + 1) * P],
                                    rhs=wt,
                                    start=(kt == 0),
                                    stop=(kt == KT_L - 1),
                                )
                        for si in range(4):
                            st = sg * 4 + si
                            nc.scalar.copy(
                                v_sb[:, st, hlc * QCW:(hlc + 1) * QCW], ps[si]
                            )

            # ---------- phase 5: KT (rope per head, contract latT over LAT)
            with (
                tc.tile_pool(name="ws3", bufs=4) as ws1,
                tc.tile_pool(name="rope3", bufs=3) as rp1,
            ):
                # KT: rope per head (contract latT over LAT)
                for og in range(4):
                    ps = proj_og(wkup, latT_sb, KT_L, og, "wk")
                    if og == 0:
                        # attention-phase constants join the DMA queue here
                        nc.sync.dma_start(mask_sb, maskTd)
                        nc.sync.dma_start(ones_col, onescd.bitcast(F32R))
                        nc.sync.dma_start(ones_k, oneskd.bitcast(F32R))
                    for oi in range(2):
                        h = og * 2 + oi
                        raw = rp1.tile([P, S], F32, tag="raw", name="raw")
                        for ntc in range(NQC):
                            nc.scalar.copy(
                                raw[:, ntc * QCW:(ntc + 1) * QCW], ps[(oi, ntc)]
                            )
                        _rope(nc, rp1, raw, kT_sb[:, h, :], cosT_sb, sinTe_sb,
                              dve_sin=(oi == 0), dma_shift=True)

            pacc_cm.__exit__(None, None, None)  # free PSUM for attention pools

            # ---------- attention + output projection
            with (
                tc.tile_pool(name="ctxp", bufs=1) as ctxp,
                tc.tile_pool(name="exl", bufs=8) as exl,
                tc.tile_pool(name="small", bufs=3) as small,
                tc.tile_pool(name="pctx", bufs=2, space="PSUM") as pctx,
                tc.tile_pool(name="psum1", bufs=2, space="PSUM") as psum1,
                tc.tile_pool(name="pbcpo", bufs=2, space="PSUM") as pbcpo,
                tc.tile_pool(name="wos", bufs=4) as wos,
                tc.tile_pool(name="outsb", bufs=3) as outsb,
            ):
                ctxT_sb = ctxp.tile([P, NHL, S], F32R)

                def finalize(fin):
                    ctx_f, sums_f, h_f, qc_f = fin
                    srow = small.tile([1, QCW], F32R, tag="srow", name="srow")
                    nc.any.tensor_copy(srow, sums_f)
                    bc = pbcpo.tile([P, QCW], F32, tag="bcpo", name="bc")
                    nc.tensor.matmul(
                        bc, lhsT=ones_col, rhs=srow, start=True, stop=True
                    )
                    rec = small.tile([P, QCW], F32, tag="rec", name="rec")
                    nc.vector.reciprocal(out=rec, in_=bc)
                    nc.vector.tensor_mul(
                        ctxT_sb[:, h_f, qc_f * QCW:(qc_f + 1) * QCW], ctx_f, rec
                    )

                pending = None
                for qc in range(NQC):
                    for h in range(NHL):
                        nkt = 4 * qc + 4  # k-tiles covering causal range
                        ctx = pctx.tile([P, QCW], F32, tag="ctx")
                        sums = psum1.tile([1, QCW], F32, tag="sums")

                        def block_geom(kt):
                            """(lo, w, mask_ap): sliced q-range for causal."""
                            off = kt - 4 * qc
                            if off < 0:
                                return 0, QCW, None
                            if off < 3:
                                # triangle sits in the first 128 sliced cols
                                return 128 * off, QCW - 128 * off, \
                                    mask_sb[:, 0:P]
                            # off == 3: keep moving width >= 256 for fp32r
                            return 256, 256, mask_sb[:, P:3 * P]

                        def emit_sc(kt):
                            lo, w, mk = block_geom(kt)
                            sc = psc.tile([P, QCW], F32, tag="sc", name="sc")
                            nc.tensor.matmul(
                                sc[:, :w],
                                lhsT=kT_sb[:, h, kt * P:(kt + 1) * P],
                                rhs=qT_sb[:, h,
                                          qc * QCW + lo:qc * QCW + lo + w],
                                start=True,
                                stop=True,
                            )
                            ex = exl.tile([P, QCW], F32R, tag="ex", name="ex")
                            nc.scalar.activation(
                                out=ex[:, :w], in_=sc[:, :w],
                                func=mybir.ActivationFunctionType.Exp,
                                scale=SC_SCALE,
                            )
                            if mk is not None:  # causal zeroing of the triangle
                                mw = mk.shape[-1]
                                eng = nc.vector if (kt % 2) else nc.gpsimd
                                eng.tensor_mul(ex[:, :mw], ex[:, :mw], mk)
                            return ex

                        def emit_pv(kt, ex):
                            lo, w, _ = block_geom(kt)
                            nc.tensor.matmul(
                                ctx[:, lo:lo + w],
                                lhsT=v_sb[:, kt, h * P:(h + 1) * P],
                                rhs=ex[:, :w],
                                start=(kt == 0),
                                stop=(kt == nkt - 1),
                            )
                            nc.tensor.matmul(
                                sums[:, lo:lo + w],
                                lhsT=ones_k,
                                rhs=ex[:, :w],
                                start=(kt == 0),
                                stop=(kt == nkt - 1),
                            )

                        # software-pipelined emission: sc(kt+1) before pv(kt)
                        exs = {0: emit_sc(0)}
                        for kt in range(nkt):
                            if kt + 1 < nkt:
                                exs[kt + 1] = emit_sc(kt + 1)
                            emit_pv(kt, exs.pop(kt))
                        # normalize the PREVIOUS head so its srow copy has a
                        # whole head of PE work to hide behind
                        if pending is not None:
                            finalize(pending)
                        pending = (ctx, sums, h, qc)
                finalize(pending)

                # out-projection: outT[o, s] = sum_hl Wo[hl, o] * ctxT[hl, s]
                for ot in range(HID // P):
                    wt = wos.tile([P, NHL, P], F32R, tag="wo", name="wt")
                    nc.sync.dma_start(
                        wt,
                        wo[:, ot * P:(ot + 1) * P].rearrange(
                            "(kt p) o -> p kt o", p=P
                        ).bitcast(F32R),
                    )
                    for qc in range(NQC):
                        po = pbcpo.tile([P, QCW], F32, tag="bcpo", name="po")
                        for kt in range(NHL):
                            nc.tensor.matmul(
                                po,
                                lhsT=wt[:, kt, :],
                                rhs=ctxT_sb[:, kt, qc * QCW:(qc + 1) * QCW],
                                start=(kt == 0),
                                stop=(kt == NHL - 1),
                            )
                        osb = outsb.tile([P, QCW], F32, tag="osb")
                        nc.any.tensor_copy(osb, po)
                        nc.sync.dma_start(
                            outT[ot * P:(ot + 1) * P, qc * QCW:(qc + 1) * QCW],
                            osb,
                        )
            vpool_cm.__exit__(None, None, None)
    nc.compile()
    return nc


# ---------------- host side ----------------

def _host_consts():
    inv_freq = 1.0 / (10000.0 ** (np.arange(0, HD, 2, dtype=np.float64) / HD))
    t = np.arange(S, dtype=np.float64)
    freqs = t[:, None] * inv_freq[None, :]            # [S, 64]
    emb = np.concatenate([freqs, freqs], axis=-1)     # [S, 128]
    cosT = np.cos(emb).T.astype(np.float32).copy()    # [128, S]
    sinT = np.sin(emb).T.astype(np.float32)
    sinTe = sinT.copy()
    sinTe[:64] *= -1.0                                # sign of rotate_half folded in
    sinTe = np.ascontiguousarray(sinTe.astype(np.float32))

    ii = np.arange(P)[:, None]
    tri = (np.arange(P)[None, :] - ii >= 0).astype(np.float32)       # [128,128]
    maskb = np.concatenate([np.zeros((P, P), np.float32), tri], axis=1)
    maskT = np.ascontiguousarray(np.concatenate([tri, maskb], axis=1))  # [128,384]
    return cosT, sinTe, maskT


_CACHE = {}


def _get_built():
    if "nc" not in _CACHE:
        _CACHE["nc"] = build_bass()
        _CACHE["consts"] = _host_consts()
    return _CACHE["nc"], _CACHE["consts"]


def make_in_maps(x, Wq, bq, Wdown, Wk_up, Wv_up, Wo):
    cosT, sinTe, maskT = _get_built()[1]
    in_maps = []
    for c in range(N_CORES):
        b, hg = c // 2, c % 2
        sl = slice(hg * HL, (hg + 1) * HL)
        in_maps.append({
            "xT": np.ascontiguousarray(x[b].T),
            "wq": np.ascontiguousarray(Wq[:, sl]),
            "wdown": np.ascontiguousarray(Wdown),
            "wkup": np.ascontiguousarray(Wk_up[:, sl]),
            "wvup": np.ascontiguousarray(Wv_up[:, sl]),
            "wo": np.ascontiguousarray(Wo[sl, :]),
            "bq": np.ascontiguousarray(bq[sl].reshape(NHL, P).T),
            "cosT": cosT,
            "sinTe": sinTe,
            "maskT": maskT,
            "ones_c": np.ones((1, P), np.float32),
            "ones_r": np.ones((P, 1), np.float32),
        })
    return in_maps


def gather_out(results, bo):
    out = np.empty((B, S, HID), dtype=np.float32)
    for b in range(B):
        acc = results[2 * b]["outT"] + results[2 * b + 1]["outT"]  # [HID, S]
        out[b] = acc.T + bo[None, :]
    return out


def kernel(x, Wq, bq, Wdown, Wk_up, Wv_up, Wo, bo):
    x = np.asarray(x, dtype=np.float32)
    Wq = np.asarray(Wq, dtype=np.float32)
    bq = np.asarray(bq, dtype=np.float32)
    Wdown = np.asarray(Wdown, dtype=np.float32)
    Wk_up = np.asarray(Wk_up, dtype=np.float32)
    Wv_up = np.asarray(Wv_up, dtype=np.float32)
    Wo = np.asarray(Wo, dtype=np.float32)
    bo = np.asarray(bo, dtype=np.float32)

    nc, _ = _get_built()
    in_maps = make_in_maps(x, Wq, bq, Wdown, Wk_up, Wv_up, Wo)
    res = run_bass_kernel_spmd(nc, in_maps, core_ids=list(range(N_CORES)))
    return gather_out(res.results, bo)



# revision 3
# speedup vs baseline: 1.0271x; 1.0271x over previous
"""Trainium2 Bass kernel: MultiHeadLatentAttention (bf16 pipeline).

Problem (hardcoded): B=4, S=1024, HID=2048, NH=16 heads of HD=128, LAT=512,
fp32 in/out, causal attention with RoPE, latent-compressed K/V (MLA).

Sharding over 8 NeuronCores: core c = (batch b = c//2, head-group hg = c%2).
Each core handles one batch element and 8 heads (local width HL=1024).

All matmul operands are bf16 (host casts); PSUM accumulation is fp32.
Verified numerically: bf16-everywhere gives ~5e-3 max-rel vs the 2e-2 gate.

Device layout (contraction dim always on SBUF partitions):
  xT      [P, 16, S]  bf16 (host pre-swizzled x[b].T)
  QT = (x Wq + bq).T  -> qT [P, 8, S]   per head h: qT[:, h, :] = q_h.T
  latT = (x Wdown).T  -> [P, 4, S]
  KT = (lat Wk_up).T  -> kT [P, 8, S]
  V  natural          -> v  [P, 8, HL]  ([s%128, s//128, hl])
  RoPE per head-pair on [P, 2, S] tiles:
      out = raw*cos2 + shift64(raw)*sin2e; the partition shift is done with
      two SBUF->SBUF DMAs (cross-partition copies are slow on compute
      engines); sign of rotate_half folded into sin2e by the host.
  scoresT_h = k_h @ q_h.T  [k, q] blocks; diagonal blocks column-sliced to
      widths 512/384/256/128 and the residual triangle zeroed by a tri mask.
  ex = exp(scores/sqrt(128)) in bf16
  sums: per half-group of 4 heads, one PSUM tile [4, 512] accumulates
      sel-ones matmuls (lhsT column h = ones) -> one reciprocal for 4 heads.
  ctxT unnormalized -> SBUF bf16; normalized by bc = sel4^T @ rec (PSUM
      broadcast matmul) with a vector multiply.
  outT_partial = Wo_s.T @ ctxT -> [HID, S] fp32, out-proj of q-chunk 0
      interleaved into the attention of q-chunk 1.

Host gathers: out[b] = (outT[2b] + outT[2b+1]).T + bo.
"""

import os

if "axon" not in os.environ.get("JAX_PLATFORMS", ""):
    os.environ["JAX_PLATFORMS"] = "axon"

import contextlib

import ml_dtypes
import numpy as np

import concourse.bacc as bacc
import concourse.mybir as mybir
import concourse.tile as tile
from concourse.bass_utils import run_bass_kernel_spmd

# ---- problem dims (hardcoded per contest rules)
B, S, HID, NH, LAT = 4, 1024, 2048, 16, 512
HD = 128
NHL = NH // 2          # heads per core = 8
HL = NHL * HD          # local head width = 1024
P = 128
KT_H = HID // P        # 16
KT_L = LAT // P        # 4
QCW = 512              # q-chunk width (PSUM bank = 512 fp32)
NQC = S // QCW         # 2
SC_SCALE = float(1.0 / np.sqrt(HD))

F32 = mybir.dt.float32
BF16 = mybir.dt.bfloat16
NPBF = ml_dtypes.bfloat16

N_CORES = 8


def build_bass(loop_iters=None):
    nc = bacc.Bacc("TRN2", target_bir_lowering=False, debug=False, num_devices=8)

    xTd = nc.dram_tensor("xT", [P, KT_H, S], BF16, kind="ExternalInput")[:]
    wqd = nc.dram_tensor("wq", [P, KT_H, HL], BF16, kind="ExternalInput")[:]
    wdownd = nc.dram_tensor("wdown", [P, KT_H, LAT], BF16, kind="ExternalInput")[:]
    wkupd = nc.dram_tensor("wkup", [P, KT_L, HL], BF16, kind="ExternalInput")[:]
    wvupd = nc.dram_tensor("wvup", [P, KT_L, HL], BF16, kind="ExternalInput")[:]
    wod = nc.dram_tensor("wo", [P, NHL, HID], BF16, kind="ExternalInput")[:]
    bqd = nc.dram_tensor("bq", [P, NHL], F32, kind="ExternalInput")[:]
    cos2d = nc.dram_tensor("cos2", [P, 2, S], BF16, kind="ExternalInput")[:]
    sin2d = nc.dram_tensor("sin2", [P, 2, S], BF16, kind="ExternalInput")[:]
    trid = nc.dram_tensor("tri", [P, P], BF16, kind="ExternalInput")[:]
    selod = nc.dram_tensor("selones", [P, 16], BF16, kind="ExternalInput")[:]
    sel4d = nc.dram_tensor("sel4", [4, 4 * P], BF16, kind="ExternalInput")[:]
    outTd = nc.dram_tensor("outT", [HID, S], F32, kind="ExternalOutput")[:]

    with tile.TileContext(nc) as tc, contextlib.ExitStack() as _les:
        if loop_iters is not None:
            _les.enter_context(tc.For_i(0, loop_iters, 1))
        with (
            tc.tile_pool(name="consts", bufs=1) as consts,
            tc.tile_pool(name="resident", bufs=1) as resident,
        ):
            cos2_sb = consts.tile([P, 2, S], BF16)
            sin2_sb = consts.tile([P, 2, S], BF16)
            tri_sb = consts.tile([P, P], BF16)
            bq_sb = consts.tile([P, NHL], F32)
            selo_sb = consts.tile([P, 16], BF16)
            sel4_sb = consts.tile([4, 4 * P], BF16)

            latT = resident.tile([P, KT_L, S], BF16)
            qT = resident.tile([P, NHL, S], BF16)
            kT = resident.tile([P, NHL, S], BF16)
            v_sb = resident.tile([P, NHL, HL], BF16)
            ctxT = resident.tile([P, NHL, S], BF16)

            pacc_cm = tc.tile_pool(name="pacc", bufs=8, space="PSUM")
            pacc = pacc_cm.__enter__()

            def rope_pair(rp, h, ps4, dst, bias):
                """RoPE for heads h, h+1 from 4 psum tiles [(j,ntc)]."""
                raw = rp.tile([P, 2, S], BF16, tag="raw", name="raw")
                sh = rp.tile([P, 2, S], BF16, tag="sh", name="sh")
                for j in range(2):
                    for ntc in range(NQC):
                        seg = raw[:, j, ntc * QCW:(ntc + 1) * QCW]
                        eng = nc.scalar if ntc == 0 else nc.vector
                        if bias:
                            nc.scalar.add(seg, ps4[j * 2 + ntc],
                                          bq_sb[:, h + j:h + j + 1])
                        else:
                            if ntc == 0:
                                nc.scalar.copy(seg, ps4[j * 2 + ntc])
                            else:
                                nc.vector.tensor_copy(seg, ps4[j * 2 + ntc])
                nc.sync.dma_start(sh[0:64, :, :], raw[64:128, :, :])
                nc.sync.dma_start(sh[64:128, :, :], raw[0:64, :, :])
                out = dst[:, h:h + 2, :]
                nc.vector.tensor_mul(out, raw, cos2_sb)
                sin_eng = nc.gpsimd if (h // 2) % 2 == 0 else nc.vector
                sin_eng.tensor_mul(sh, sh, sin2_sb)
                nc.vector.tensor_add(out, out, sh)

            # ---------- phase A: QT (2 groups of 4 heads) + latT ----------
            with (
                tc.tile_pool(name="xp", bufs=1) as xp,
                tc.tile_pool(name="ws1", bufs=4) as ws1,
                tc.tile_pool(name="ropeA", bufs=2) as rpA,
            ):
                xT_sb = xp.tile([P, KT_H, S], BF16)
                nc.sync.dma_start(bq_sb, bqd)

                for og in range(2):
                    ps = [pacc.tile([P, QCW], F32, tag="acc", name="acc")
                          for _ in range(8)]
                    for kt in range(KT_H):
                        if og == 0:
                            nc.sync.dma_start(xT_sb[:, kt, :], xTd[:, kt, :])
                        wt = ws1.tile([P, 4 * P], BF16, tag="w", name="wt")
                        nc.sync.dma_start(
                            wt, wqd[:, kt, og * 4 * P:(og + 1) * 4 * P])
                        for oi in range(4):
                            for ntc in range(NQC):
                                nc.tensor.matmul(
                                    ps[oi * 2 + ntc],
                                    lhsT=wt[:, oi * P:(oi + 1) * P],
                                    rhs=xT_sb[:, kt,
                                              ntc * QCW:(ntc + 1) * QCW],
                                    start=(kt == 0),
                                    stop=(kt == KT_H - 1),
                                )
                    if og == 0:
                        nc.sync.dma_start(cos2_sb, cos2d)
                        nc.sync.dma_start(sin2_sb, sin2d)
                        nc.sync.dma_start(tri_sb, trid)
                        nc.sync.dma_start(selo_sb, selod)
                        nc.sync.dma_start(sel4_sb, sel4d)
                    for pr in range(2):
                        rope_pair(rpA, og * 4 + pr * 2,
                                  ps[pr * 4:pr * 4 + 4], qT, bias=True)

                # latT group (4 out tiles x 2 chunks)
                ps = [pacc.tile([P, QCW], F32, tag="acc", name="acc")
                      for _ in range(8)]
                for kt in range(KT_H):
                    wt = ws1.tile([P, LAT], BF16, tag="wd", name="wt")
                    nc.sync.dma_start(wt, wdownd[:, kt, :])
                    for oi in range(4):
                        for ntc in range(NQC):
                            nc.tensor.matmul(
                                ps[oi * 2 + ntc],
                                lhsT=wt[:, oi * P:(oi + 1) * P],
                                rhs=xT_sb[:, kt, ntc * QCW:(ntc + 1) * QCW],
                                start=(kt == 0),
                                stop=(kt == KT_H - 1),
                            )
                for oi in range(4):
                    for ntc in range(NQC):
                        eng = nc.scalar if ntc == 0 else nc.vector
                        if ntc == 0:
                            nc.scalar.copy(
                                latT[:, oi, ntc * QCW:(ntc + 1) * QCW],
                                ps[oi * 2 + ntc])
                        else:
                            nc.vector.tensor_copy(
                                latT[:, oi, ntc * QCW:(ntc + 1) * QCW],
                                ps[oi * 2 + ntc])

            # ---------- phase B: V natural + KT (contract latT) ----------
            wop_cm = tc.tile_pool(name="wop", bufs=1)
            wop = wop_cm.__enter__()
            wo_sb = wop.tile([P, NHL, HID], BF16)
            nc.sync.dma_start(wo_sb, wod)

            with (
                tc.tile_pool(name="ws2", bufs=4) as ws2,
                tc.tile_pool(name="ropeB", bufs=2) as rpB,
            ):
                for hlc in range(2):
                    ps = [pacc.tile([P, QCW], F32, tag="acc", name="acc")
                          for _ in range(8)]
                    for kt in range(KT_L):
                        wt = ws2.tile([P, QCW], BF16, tag="wv", name="wt")
                        nc.sync.dma_start(
                            wt, wvupd[:, kt, hlc * QCW:(hlc + 1) * QCW])
                        for st in range(8):
                            nc.tensor.matmul(
                                ps[st],
                                lhsT=latT[:, kt, st * P:(st + 1) * P],
                                rhs=wt,
                                start=(kt == 0),
                                stop=(kt == KT_L - 1),
                            )
                    for st in range(8):
                        if st % 2 == 0:
                            nc.scalar.copy(
                                v_sb[:, st, hlc * QCW:(hlc + 1) * QCW], ps[st])
                        else:
                            nc.vector.tensor_copy(
                                v_sb[:, st, hlc * QCW:(hlc + 1) * QCW], ps[st])

                for og in range(2):
                    ps = [pacc.tile([P, QCW], F32, tag="acc", name="acc")
                          for _ in range(8)]
                    for kt in range(KT_L):
                        wt = ws2.tile([P, 4 * P], BF16, tag="wk", name="wt")
                        nc.sync.dma_start(
                            wt, wkupd[:, kt, og * 4 * P:(og + 1) * 4 * P])
                        for oi in range(4):
                            for ntc in range(NQC):
                                nc.tensor.matmul(
                                    ps[oi * 2 + ntc],
                                    lhsT=wt[:, oi * P:(oi + 1) * P],
                                    rhs=latT[:, kt, ntc * QCW:(ntc + 1) * QCW],
                                    start=(kt == 0),
                                    stop=(kt == KT_L - 1),
                                )
                    for pr in range(2):
                        rope_pair(rpB, og * 4 + pr * 2,
                                  ps[pr * 4:pr * 4 + 4], kT, bias=False)

            pacc_cm.__exit__(None, None, None)

            # ---------- phase C: attention + out-projection ----------
            with (
                tc.tile_pool(name="psc", bufs=2, space="PSUM") as psc,
                tc.tile_pool(name="pctx", bufs=2, space="PSUM") as pctx,
                tc.tile_pool(name="psums", bufs=2, space="PSUM") as psums,
                tc.tile_pool(name="pbc", bufs=2, space="PSUM") as pbc,
                tc.tile_pool(name="exl", bufs=6) as exl,
                tc.tile_pool(name="small", bufs=2) as small,
                tc.tile_pool(name="outsb", bufs=3) as outsb,
            ):
                def emit_outproj_tile(qc, ot):
                    po = pbc.tile([P, QCW], F32, tag="bcpo", name="po")
                    for kt in range(NHL):
                        nc.tensor.matmul(
                            po,
                            lhsT=wo_sb[:, kt, ot * P:(ot + 1) * P],
                            rhs=ctxT[:, kt, qc * QCW:(qc + 1) * QCW],
                            start=(kt == 0),
                            stop=(kt == NHL - 1),
                        )
                    ob = outsb.tile([P, QCW], F32, tag="osb", name="ob")
                    if ot % 2 == 0:
                        nc.scalar.copy(ob, po)
                    else:
                        nc.vector.tensor_copy(ob, po)
                    nc.sync.dma_start(
                        outTd[ot * P:(ot + 1) * P, qc * QCW:(qc + 1) * QCW],
                        ob)

                pending_op = []  # deferred out-proj tile emitters

                def attention_head(h, hh, qc, nkt, ctx, sums_ps):
                    def geom(kt):
                        off = kt - 4 * qc
                        if off < 0:
                            return 0, QCW, False
                        return 128 * off, QCW - 128 * off, True

                    def emit_sc(kt):
                        lo, w, diag = geom(kt)
                        sc = psc.tile([P, QCW], F32, tag="sc", name="sc")
                        nc.tensor.matmul(
                            sc[:, :w],
                            lhsT=kT[:, h, kt * P:(kt + 1) * P],
                            rhs=qT[:, h, qc * QCW + lo:qc * QCW + lo + w],
                            start=True, stop=True,
                        )
                        ex = exl.tile([P, QCW], BF16, tag="ex", name="ex")
                        nc.scalar.activation(
                            out=ex[:, :w], in_=sc[:, :w],
                            func=mybir.ActivationFunctionType.Exp,
                            scale=SC_SCALE,
                        )
                        if diag:
                            eng = nc.vector if (kt % 2) else nc.gpsimd
                            eng.tensor_mul(ex[:, 0:P], ex[:, 0:P], tri_sb)
                        return ex

                    def emit_pv(kt, ex):
                        lo, w, _ = geom(kt)
                        nc.tensor.matmul(
                            ctx[:, lo:lo + w],
                            lhsT=v_sb[:, kt, h * P:(h + 1) * P],
                            rhs=ex[:, :w],
                            start=(kt == 0),
                            stop=(kt == nkt - 1),
                        )
                        nc.tensor.matmul(
                            sums_ps[:, lo:lo + w],
                            lhsT=selo_sb[:, hh * 4:(hh + 1) * 4],
                            rhs=ex[:, :w],
                            start=(hh == 0 and kt == 0),
                            stop=(hh == 3 and kt == nkt - 1),
                        )

                    exs = {0: emit_sc(0)}
                    for kt in range(nkt):
                        if kt + 1 < nkt:
                            exs[kt + 1] = emit_sc(kt + 1)
                        emit_pv(kt, exs.pop(kt))

                for qc in range(NQC):
                    nkt = 4 * qc + 4
                    for half in range(2):
                        sums_ps = psums.tile([4, QCW], F32, tag="sums",
                                             name="sums")
                        for hh in range(4):
                            h = half * 4 + hh
                            ctx = pctx.tile([P, QCW], F32, tag="ctx",
                                            name="ctx")
                            attention_head(h, hh, qc, nkt, ctx, sums_ps)
                            nc.vector.tensor_copy(
                                ctxT[:, h, qc * QCW:(qc + 1) * QCW], ctx)
                            for _ in range(2):
                                if pending_op:
                                    pending_op.pop(0)()
                        rec = small.tile([4, QCW], BF16, tag="rec",
                                         name="rec")
                        with nc.allow_low_precision(reason="bf16 softmax "
                                                    "denominator (gate 2e-2)"):
                            nc.vector.reciprocal(out=rec, in_=sums_ps)
                        for hh in range(4):
                            h = half * 4 + hh
                            bc = pbc.tile([P, QCW], F32, tag="bcpo",
                                          name="bc")
                            nc.tensor.matmul(
                                bc,
                                lhsT=sel4_sb[:, hh * P:(hh + 1) * P],
                                rhs=rec,
                                start=True, stop=True,
                            )
                            sl = ctxT[:, h, qc * QCW:(qc + 1) * QCW]
                            nc.vector.tensor_mul(sl, sl, bc)
                    for ot in range(HID // P):
                        pending_op.append(
                            (lambda q_, o_: lambda: emit_outproj_tile(q_, o_))
                            (qc, ot))
                while pending_op:
                    pending_op.pop(0)()

            wop_cm.__exit__(None, None, None)
    nc.compile()
    return nc


# ---------------- host side ----------------

def _host_consts():
    inv_freq = 1.0 / (10000.0 ** (np.arange(0, HD, 2, dtype=np.float64) / HD))
    t = np.arange(S, dtype=np.float64)
    freqs = t[:, None] * inv_freq[None, :]            # [S, 64]
    emb = np.concatenate([freqs, freqs], axis=-1)     # [S, 128]
    cosT = np.cos(emb).T.astype(np.float32)           # [128, S]
    sinT = np.sin(emb).T.astype(np.float32)
    sinTe = sinT.copy()
    sinTe[:64] *= -1.0                                # rotate_half sign folded
    cos2 = np.ascontiguousarray(
        np.broadcast_to(cosT[:, None, :], (P, 2, S))).astype(NPBF)
    sin2 = np.ascontiguousarray(
        np.broadcast_to(sinTe[:, None, :], (P, 2, S))).astype(NPBF)

    ii = np.arange(P)[:, None]
    tri = (np.arange(P)[None, :] - ii >= 0).astype(NPBF)  # [128,128]

    selones = np.zeros((P, 16), dtype=NPBF)
    for hh in range(4):
        selones[:, hh * 4 + hh] = 1.0
    sel4 = np.zeros((4, 4 * P), dtype=NPBF)
    for hh in range(4):
        sel4[hh, hh * P:(hh + 1) * P] = 1.0
    return cos2, sin2, tri, selones, sel4


_CACHE = {}


def _get_built():
    if "nc" not in _CACHE:
        _CACHE["nc"] = build_bass()
        _CACHE["consts"] = _host_consts()
    return _CACHE["nc"], _CACHE["consts"]


def _swz(a, n_kt):
    """[n_kt*128, W] -> [128, n_kt, W] (partition-major swizzle), bf16."""
    w = a.shape[1]
    return np.ascontiguousarray(
        a.reshape(n_kt, P, w).transpose(1, 0, 2)).astype(NPBF)


def make_in_maps(x, Wq, bq, Wdown, Wk_up, Wv_up, Wo):
    cos2, sin2, tri, selones, sel4 = _get_built()[1]
    in_maps = []
    for c in range(N_CORES):
        b, hg = c // 2, c % 2
        sl = slice(hg * HL, (hg + 1) * HL)
        in_maps.append({
            "xT": _swz(np.ascontiguousarray(x[b].T), KT_H),
            "wq": _swz(Wq[:, sl], KT_H),
            "wdown": _swz(Wdown, KT_H),
            "wkup": _swz(Wk_up[:, sl], KT_L),
            "wvup": _swz(Wv_up[:, sl], KT_L),
            "wo": _swz(Wo[sl, :], NHL),
            "bq": np.ascontiguousarray(
                bq[sl].reshape(NHL, P).T).astype(np.float32),
            "cos2": cos2,
            "sin2": sin2,
            "tri": tri,
            "selones": selones,
            "sel4": sel4,
        })
    return in_maps


def gather_out(results, bo):
    out = np.empty((B, S, HID), dtype=np.float32)
    for b in range(B):
        acc = results[2 * b]["outT"] + results[2 * b + 1]["outT"]  # [HID, S]
        out[b] = acc.T + bo[None, :]
    return out


def kernel(x, Wq, bq, Wdown, Wk_up, Wv_up, Wo, bo):
    x = np.asarray(x, dtype=np.float32)
    Wq = np.asarray(Wq, dtype=np.float32)
    bq = np.asarray(bq, dtype=np.float32)
    Wdown = np.asarray(Wdown, dtype=np.float32)
    Wk_up = np.asarray(Wk_up, dtype=np.float32)
    Wv_up = np.asarray(Wv_up, dtype=np.float32)
    Wo = np.asarray(Wo, dtype=np.float32)
    bo = np.asarray(bo, dtype=np.float32)

    nc, _ = _get_built()
    in_maps = make_in_maps(x, Wq, bq, Wdown, Wk_up, Wv_up, Wo)
    res = run_bass_kernel_spmd(nc, in_maps, core_ids=list(range(N_CORES)))
    return gather_out(res.results, bo)


# revision 22
# speedup vs baseline: 1.5801x; 1.5383x over previous
"""Trainium2 Bass kernel: MultiHeadLatentAttention (bf16 pipeline).

Problem (hardcoded): B=4, S=1024, HID=2048, NH=16 heads of HD=128, LAT=512,
fp32 in/out, causal attention with RoPE, latent-compressed K/V (MLA).

Sharding over 8 NeuronCores: core c = (batch b = c//2, head-group hg = c%2).
Each core handles one batch element and 8 heads (local width HL=1024).

All matmul operands are bf16 (host casts); PSUM accumulation is fp32
(bf16-everywhere measures ~5e-3 max-rel vs the 2e-2 gate).

Device layout (contraction dim always on SBUF partitions; all SBUF tiles
flat 2D [128, cols]):
  xT   [P, 16*S] bf16 (host pre-swizzled x[b].T), 4 batched DMAs
  QT = (x Wq + bq).T -> qT [P, 8*S];  latT = (x Wdown).T -> [P, 4*S]
  KT = (lat Wk_up).T -> kT [P, 8*S];  V natural -> v [P, 8*HL]
  RoPE per head-pair on [P, 2S] tiles: out = raw*cos2 + shift64(raw)*sin2e;
    the partition shift is two SBUF->SBUF DMAs issued from the SCALAR queue
    (HWDGE) so they never head-of-line-block the weight stream on Sync.
  scoresT_h = k_h @ q_h.T in [k,q] blocks; diagonal blocks column-sliced to
    widths 512/384/256/128, residual triangle zeroed by a tri mask.
  ex = exp(scores/sqrt(128)) bf16
  sums: per half-group of 4 heads one PSUM tile [4,512] accumulates
    sel-ones matmuls -> one reciprocal serves 4 heads.
  ctxT unnormalized bf16; normalized via bc = sel4^T @ rec broadcast matmul.
  out-proj of q-chunk 0 interleaved into attention of q-chunk 1.

DMA issue budget: weights/x batched into ~1MB transfers on Sync; rope
shifts + half the outT stores on Scalar (second HWDGE queue).

Host gathers: out[b] = (outT[2b] + outT[2b+1]).T + bo.
"""

import os

if "axon" not in os.environ.get("JAX_PLATFORMS", ""):
    os.environ["JAX_PLATFORMS"] = "axon"

import contextlib

import ml_dtypes
import numpy as np

import concourse.bacc as bacc
import concourse.mybir as mybir
import concourse.tile as tile
from concourse.bass_utils import run_bass_kernel_spmd

# ---- problem dims (hardcoded per contest rules)
B, S, HID, NH, LAT = 4, 1024, 2048, 16, 512
HD = 128
NHL = NH // 2          # heads per core = 8
HL = NHL * HD          # local head width = 1024
P = 128
KT_H = HID // P        # 16
KT_L = LAT // P        # 4
QCW = 512              # q-chunk width (PSUM bank = 512 fp32)
NQC = S // QCW         # 2
SC_SCALE = float(1.0 / np.sqrt(HD))

F32 = mybir.dt.float32
BF16 = mybir.dt.bfloat16
NPBF = ml_dtypes.bfloat16

N_CORES = 8
CPACK_W = 2 * S + 2 * S + P + 4 * P   # cos2 | sin2 | tri | selones128


def build_bass(loop_iters=None):
    nc = bacc.Bacc("TRN2", target_bir_lowering=False, debug=False, num_devices=8)

    xTd = nc.dram_tensor("xT", [P, KT_H, S], BF16, kind="ExternalInput")[:]
    wqd = nc.dram_tensor("wq", [P, KT_H, HL], BF16, kind="ExternalInput")[:]
    wdownd = nc.dram_tensor("wdown", [P, KT_H, LAT], BF16, kind="ExternalInput")[:]
    wkupd = nc.dram_tensor("wkup", [P, KT_L, HL], BF16, kind="ExternalInput")[:]
    wvupd = nc.dram_tensor("wvup", [P, KT_L, HL], BF16, kind="ExternalInput")[:]
    wod = nc.dram_tensor("wo", [P, NHL, HID], BF16, kind="ExternalInput")[:]
    bqd = nc.dram_tensor("bq", [P, NHL], F32, kind="ExternalInput")[:]
    cpackd = nc.dram_tensor("cpack", [P, CPACK_W], BF16, kind="ExternalInput")[:]
    sel4d = nc.dram_tensor("sel4", [4, 4 * P], BF16, kind="ExternalInput")[:]
    outTd = nc.dram_tensor("outT", [HID, S], F32, kind="ExternalOutput")[:]

    with tile.TileContext(nc) as tc, contextlib.ExitStack() as _les:
        if loop_iters is not None:
            _les.enter_context(tc.For_i(0, loop_iters, 1))
        with (
            tc.tile_pool(name="consts", bufs=1) as consts,
            tc.tile_pool(name="resident", bufs=1) as resident,
        ):
            cpack = consts.tile([P, CPACK_W], BF16)
            cos2_sb = cpack[:, 0:2 * S]
            sin2_sb = cpack[:, 2 * S:4 * S]
            tri_sb = cpack[:, 4 * S:4 * S + P]
            selo_sb = cpack[:, 4 * S + P:4 * S + P + 4 * P]
            bq_sb = consts.tile([P, NHL], F32)
            sel4_sb = consts.tile([4, 4 * P], BF16)

            latT = resident.tile([P, KT_L * S], BF16)
            qT = resident.tile([P, NHL * S], BF16)
            kT = resident.tile([P, NHL * S], BF16)
            v_sb = resident.tile([P, NHL * HL], BF16)
            ctxT = resident.tile([P, NHL * S], BF16)
            # phase-B weights, loaded during phase A (wvg also feeds the
            # V hl-half-1 filler inside phase C)
            wkg = resident.tile([P, KT_L * HL], BF16)
            wvg = resident.tile([P, KT_L * HL], BF16)

            pacc_cm = tc.tile_pool(name="pacc", bufs=8, space="PSUM")
            pacc = pacc_cm.__enter__()

            def rope_pair(rp, h, ps4, dst, bias, sin_eng,
                          add_eng=None, dma_eng=None):
                """RoPE for heads h, h+1 from 4 psum tiles [(j,ntc)]."""
                add_eng = add_eng or nc.vector
                dma_eng = dma_eng or nc.scalar
                raw = rp.tile([P, 2 * S], BF16, tag="raw", name="raw")
                sh = rp.tile([P, 2 * S], BF16, tag="sh", name="sh")
                for j in range(2):
                    for ntc in range(NQC):
                        seg = raw[:, (j * NQC + ntc) * QCW:
                                  (j * NQC + ntc + 1) * QCW]
                        if bias:
                            nc.scalar.add(seg, ps4[j * 2 + ntc],
                                          bq_sb[:, h + j:h + j + 1])
                        else:
                            nc.scalar.copy(seg, ps4[j * 2 + ntc])
                    # per-head shift: unblocks as soon as this head's two
                    # segment copies land (not the whole pair)
                    dma_eng.dma_start(sh[0:64, j * S:(j + 1) * S],
                                      raw[64:128, j * S:(j + 1) * S])
                    dma_eng.dma_start(sh[64:128, j * S:(j + 1) * S],
                                      raw[0:64, j * S:(j + 1) * S])
                out = dst[:, h * S:(h + 2) * S]
                nc.vector.tensor_mul(out, raw, cos2_sb)
                sin_eng.tensor_mul(sh, sh, sin2_sb)
                add_eng.tensor_add(out, out, sh)

            # ---------- phase A: QT (2 groups of 4 heads) + latT ----------
            with (
                tc.tile_pool(name="xp", bufs=1) as xp,
                tc.tile_pool(name="ws1", bufs=2) as ws1,
                tc.tile_pool(name="ropeA", bufs=2) as rpA,
            ):
                xT_sb = xp.tile([P, KT_H * S], BF16)
                # ramp-in: first x chunk + first weight chunk land ASAP so
                # the PE starts ~4us in, then the bulk streams behind them
                wg0 = ws1.tile([P, KT_H * QCW], BF16, tag="w", name="wg")
                nc.sync.dma_start(xT_sb[:, 0:2 * S], xTd[:, 0:2, :])
                nc.sync.dma_start(wg0[:, 0:2 * QCW], wqd[:, 0:2, 0:QCW])
                nc.sync.dma_start(xT_sb[:, 2 * S:8 * S], xTd[:, 2:8, :])
                nc.sync.dma_start(wg0[:, 2 * QCW:8 * QCW],
                                  wqd[:, 2:8, 0:QCW])
                nc.sync.dma_start(xT_sb[:, 8 * S:16 * S], xTd[:, 8:16, :])
                nc.sync.dma_start(wg0[:, 8 * QCW:16 * QCW],
                                  wqd[:, 8:16, 0:QCW])
                nc.sync.dma_start(bq_sb, bqd)
                # preload the exp table set while the PE is busy with
                # projections (first ACTIVATE otherwise pays ~2.7us in C)
                warm = ws1.tile([1, NHL], F32, tag="warm", name="warm")
                nc.scalar.activation(
                    out=warm, in_=bq_sb[0:1, :],
                    func=mybir.ActivationFunctionType.Exp, scale=1.0)

                for og in range(2):
                    if og == 0:
                        wg = wg0
                    else:
                        wg = ws1.tile([P, KT_H * QCW], BF16, tag="w",
                                      name="wg")
                        for hf in range(2):   # two 1MB halves
                            nc.sync.dma_start(
                                wg[:, hf * 8 * QCW:(hf + 1) * 8 * QCW],
                                wqd[:, hf * 8:(hf + 1) * 8,
                                    og * QCW:(og + 1) * QCW])
                    ps = [pacc.tile([P, QCW], F32, tag="acc", name="acc")
                          for _ in range(8)]
                    for kt in range(KT_H):
                        for oi in range(4):
                            for ntc in range(NQC):
                                nc.tensor.matmul(
                                    ps[oi * 2 + ntc],
                                    lhsT=wg[:, kt * QCW + oi * P:
                                            kt * QCW + (oi + 1) * P],
                                    rhs=xT_sb[:, kt * S + ntc * QCW:
                                              kt * S + (ntc + 1) * QCW],
                                    start=(kt == 0),
                                    stop=(kt == KT_H - 1),
                                )
                    if og == 0:
                        nc.sync.dma_start(cpack, cpackd)
                        nc.sync.dma_start(sel4_sb, sel4d)
                    for pr in range(2):
                        rope_pair(rpA, og * 4 + pr * 2,
                                  ps[pr * 4:pr * 4 + 4], qT, bias=True,
                                  sin_eng=nc.gpsimd)

                # latT group (4 out tiles x 2 chunks)
                wg = ws1.tile([P, KT_H * QCW], BF16, tag="w", name="wg")
                for hf in range(2):
                    nc.sync.dma_start(
                        wg[:, hf * 8 * QCW:(hf + 1) * 8 * QCW],
                        wdownd[:, hf * 8:(hf + 1) * 8, :])
                nc.sync.dma_start(wkg, wkupd)
                nc.sync.dma_start(wvg, wvupd)
                ps = [pacc.tile([P, QCW], F32, tag="acc", name="acc")
                      for _ in range(8)]
                for kt in range(KT_H):
                    for oi in range(4):
                        for ntc in range(NQC):
                            nc.tensor.matmul(
                                ps[oi * 2 + ntc],
                                lhsT=wg[:, kt * QCW + oi * P:
                                        kt * QCW + (oi + 1) * P],
                                rhs=xT_sb[:, kt * S + ntc * QCW:
                                          kt * S + (ntc + 1) * QCW],
                                start=(kt == 0),
                                stop=(kt == KT_H - 1),
                            )
                for oi in range(4):
                    for ntc in range(NQC):
                        dstap = latT[:, oi * S + ntc * QCW:
                                     oi * S + (ntc + 1) * QCW]
                        if ntc == 0:
                            nc.scalar.copy(dstap, ps[oi * 2 + ntc])
                        else:
                            nc.vector.tensor_copy(dstap, ps[oi * 2 + ntc])

            # ---------- phase B: KT (rope) then V hl-half 0 ----------
            # (wkup/wvup were loaded during phase A; wo loads during B;
            #  V hl-half 1 is emitted later as PE filler inside qc0
            #  attention, using the pctx pool.)
            wop_cm = tc.tile_pool(name="wop", bufs=1)
            wop = wop_cm.__enter__()
            wo_sb = wop.tile([P, NHL * HID], BF16)
            nc.sync.dma_start(wo_sb, wod)

            rpB_cm = tc.tile_pool(name="ropeB", bufs=2)
            rpB = rpB_cm.__enter__()

            for og in range(2):
                ps = [pacc.tile([P, QCW], F32, tag="acc", name="acc")
                      for _ in range(8)]
                for kt in range(KT_L):
                    for oi in range(4):
                        for ntc in range(NQC):
                            nc.tensor.matmul(
                                ps[oi * 2 + ntc],
                                lhsT=wkg[:, kt * HL + og * 4 * P + oi * P:
                                         kt * HL + og * 4 * P
                                         + (oi + 1) * P],
                                rhs=latT[:, kt * S + ntc * QCW:
                                         kt * S + (ntc + 1) * QCW],
                                start=(kt == 0),
                                stop=(kt == KT_L - 1),
                            )
                for pr in range(2):
                    sin = nc.vector if og == 0 else nc.gpsimd
                    rope_pair(rpB, og * 4 + pr * 2,
                              ps[pr * 4:pr * 4 + 4], kT, bias=False,
                              sin_eng=sin, add_eng=nc.vector,
                              dma_eng=nc.sync)

            ps = [pacc.tile([P, QCW], F32, tag="acc", name="acc")
                  for _ in range(8)]
            for kt in range(KT_L):
                for st in range(8):
                    nc.tensor.matmul(
                        ps[st],
                        lhsT=latT[:, kt * S + st * P:kt * S + (st + 1) * P],
                        rhs=wvg[:, kt * HL:kt * HL + QCW],
                        start=(kt == 0),
                        stop=(kt == KT_L - 1),
                    )
            for st in range(8):
                dstap = v_sb[:, st * HL:st * HL + QCW]
                if st % 4 == 1:
                    nc.vector.tensor_copy(dstap, ps[st])
                else:
                    nc.scalar.copy(dstap, ps[st])

            rpB_cm.__exit__(None, None, None)
            pacc_cm.__exit__(None, None, None)

            # ---------- phase C: attention + out-projection ----------
            with (
                tc.tile_pool(name="psc", bufs=2, space="PSUM") as psc,
                tc.tile_pool(name="pctx", bufs=2, space="PSUM") as pctx,
                tc.tile_pool(name="psums", bufs=2, space="PSUM") as psums,
                tc.tile_pool(name="pbc", bufs=2, space="PSUM") as pbc,
                tc.tile_pool(name="exla", bufs=3) as exla,
                tc.tile_pool(name="exlb", bufs=3) as exlb,
                tc.tile_pool(name="small", bufs=2) as small,
                tc.tile_pool(name="outsb", bufs=3) as outsb,
            ):
                # ---- fine-grained PE filler: each closure emits ~one
                # always-ready matmul (out-proj accumulation step or a
                # V hl-half-1 accumulation step), popped between attention
                # dependency steps to keep the PE dense and warm.
                fill = []
                vfill = []   # V hl-half-1 steps: popped first, and force-
                             # drained before any head >= 4 reads v half 1

                def fl_pop(n):
                    for _ in range(n):
                        if vfill:
                            vfill.pop(0)()
                        elif fill:
                            fill.pop(0)()

                def add_outproj(qc, ot):
                    st_ = {}

                    def mk_mm(kt):
                        def go():
                            if kt == 0:
                                st_["po"] = pbc.tile([P, QCW], F32,
                                                     tag="bcpo", name="po")
                            nc.tensor.matmul(
                                st_["po"],
                                lhsT=wo_sb[:, kt * HID + ot * P:
                                           kt * HID + (ot + 1) * P],
                                rhs=ctxT[:, kt * S + qc * QCW:
                                         kt * S + (qc + 1) * QCW],
                                start=(kt == 0),
                                stop=(kt == NHL - 1),
                            )
                        return go

                    def fin():
                        ob = outsb.tile([P, QCW], F32, tag="osb", name="ob")
                        nc.vector.tensor_copy(ob, st_["po"])
                        nc.sync.dma_start(
                            outTd[ot * P:(ot + 1) * P,
                                  qc * QCW:(qc + 1) * QCW], ob)

                    for kt in range(NHL):
                        fill.append(mk_mm(kt))
                    fill.append(fin)

                def add_vhalf1():
                    for st in range(8):
                        st_ = {}

                        def mk_mm(kt, st, st_):
                            def go():
                                if kt == 0:
                                    st_["ps"] = pctx.tile([P, QCW], F32,
                                                          tag="ctx",
                                                          name="vac")
                                nc.tensor.matmul(
                                    st_["ps"],
                                    lhsT=latT[:, kt * S + st * P:
                                              kt * S + (st + 1) * P],
                                    rhs=wvg[:, kt * HL + QCW:
                                            kt * HL + 2 * QCW],
                                    start=(kt == 0),
                                    stop=(kt == KT_L - 1),
                                )
                            return go

                        def fin(st, st_):
                            def go():
                                dst = v_sb[:, st * HL + QCW:
                                           st * HL + 2 * QCW]
                                if st % 2 == 0:
                                    nc.scalar.copy(dst, st_["ps"])
                                else:
                                    nc.vector.tensor_copy(dst, st_["ps"])
                            return go

                        for kt in range(KT_L):
                            vfill.append(mk_mm(kt, st, st_))
                        vfill.append(fin(st, st_))

                add_vhalf1()

                def attention_head(h, hh, qc, nkt, ctx, sums_ps):
                    def geom(kt):
                        off = kt - 4 * qc
                        if off < 0:
                            return 0, QCW, False
                        return 128 * off, QCW - 128 * off, True

                    def emit_sc(kt):
                        lo, w, diag = geom(kt)
                        sc = psc.tile([P, QCW], F32, tag="sc", name="sc")
                        nc.tensor.matmul(
                            sc[:, :w],
                            lhsT=kT[:, h * S + kt * P:h * S + (kt + 1) * P],
                            rhs=qT[:, h * S + qc * QCW + lo:
                                   h * S + qc * QCW + lo + w],
                            start=True, stop=True,
                        )
                        exp_pool = exla if kt % 2 == 0 else exlb
                        ex = exp_pool.tile([P, QCW], BF16, tag="ex",
                                           name="ex")
                        nc.scalar.activation(
                            out=ex[:, :w], in_=sc[:, :w],
                            func=mybir.ActivationFunctionType.Exp,
                            scale=SC_SCALE,
                        )
                        if diag:
                            # gpsimd is busy with kT h4-7 rope early in C
                            eng = (nc.vector if (qc == 0 or kt % 2)
                                   else nc.gpsimd)
                            eng.tensor_mul(ex[:, 0:P], ex[:, 0:P], tri_sb)
                        return ex

                    def emit_pv(kt, ex):
                        lo, w, _ = geom(kt)
                        nc.tensor.matmul(
                            ctx[:, lo:lo + w],
                            lhsT=v_sb[:, kt * HL + h * P:
                                      kt * HL + (h + 1) * P],
                            rhs=ex[:, :w],
                            start=(kt == 0),
                            stop=(kt == nkt - 1),
                        )
                        nc.tensor.matmul(
                            sums_ps[:, lo:lo + w],
                            lhsT=selo_sb[:, hh * P:(hh + 1) * P],
                            rhs=ex[:, :w],
                            start=(hh == 0 and kt == 0),
                            stop=(hh == 3 and kt == nkt - 1),
                        )

                    exs = {0: emit_sc(0)}
                    for kt in range(nkt):
                        if kt + 1 < nkt:
                            exs[kt + 1] = emit_sc(kt + 1)
                        emit_pv(kt, exs.pop(kt))
                        fl_pop(1 if (qc == 1 and h >= 6) else 2)

                pending_norm = []

                def flush_norm():
                    while pending_norm:
                        pending_norm.pop(0)()

                for qc in range(NQC):
                    nkt = 4 * qc + 4
                    for half in range(2):
                        if half == 1:
                            while vfill:   # heads 4-7 read v_sb hl-half 1
                                vfill.pop(0)()
                        sums_ps = psums.tile([P, QCW], F32, tag="sums",
                                             name="sums")
                        for hh in range(4):
                            h = half * 4 + hh
                            ctx = pctx.tile([P, QCW], F32, tag="ctx",
                                            name="ctx")
                            attention_head(h, hh, qc, nkt, ctx, sums_ps)
                            ctx_dst = ctxT[:, h * S + qc * QCW:
                                           h * S + (qc + 1) * QCW]
                            if qc == 0:
                                nc.scalar.copy(ctx_dst, ctx)
                            else:
                                nc.vector.tensor_copy(ctx_dst, ctx)
                            if hh == 0:
                                # emit previous half's deferred norm now:
                                # its reciprocal overlaps this head's PE work
                                flush_norm()
                            fl_pop(2)
                        rec = small.tile([4, QCW], BF16, tag="rec",
                                         name="rec")
                        with nc.allow_low_precision(reason="bf16 softmax "
                                                    "denominator (gate 2e-2)"):
                            nc.vector.reciprocal(out=rec, in_=sums_ps[0:4, :])

                        def mk_norm(qc, half, rec):
                            def go():
                                for hh in range(4):
                                    h = half * 4 + hh
                                    bc = pbc.tile([P, QCW], F32, tag="bcpo",
                                                  name="bc")
                                    nc.tensor.matmul(
                                        bc,
                                        lhsT=sel4_sb[:, hh * P:(hh + 1) * P],
                                        rhs=rec,
                                        start=True, stop=True,
                                    )
                                    sl = ctxT[:, h * S + qc * QCW:
                                              h * S + (qc + 1) * QCW]
                                    nc.vector.tensor_mul(sl, sl, bc)
                                    fl_pop(2)
                            return go

                        pending_norm.append(mk_norm(qc, half, rec))
                    flush_norm()  # qc complete: all heads normalized
                    for ot in range(HID // P):
                        add_outproj(qc, ot)
                while fill:
                    fill.pop(0)()

            wop_cm.__exit__(None, None, None)
    nc.compile()
    return nc


# ---------------- host side ----------------

def _host_consts():
    inv_freq = 1.0 / (10000.0 ** (np.arange(0, HD, 2, dtype=np.float64) / HD))
    t = np.arange(S, dtype=np.float64)
    freqs = t[:, None] * inv_freq[None, :]            # [S, 64]
    emb = np.concatenate([freqs, freqs], axis=-1)     # [S, 128]
    cosT = np.cos(emb).T.astype(np.float32)           # [128, S]
    sinT = np.sin(emb).T.astype(np.float32)
    sinTe = sinT.copy()
    sinTe[:64] *= -1.0                                # rotate_half sign folded
    cos2 = np.broadcast_to(cosT[:, None, :], (P, 2, S)).reshape(P, 2 * S)
    sin2 = np.broadcast_to(sinTe[:, None, :], (P, 2, S)).reshape(P, 2 * S)

    ii = np.arange(P)[:, None]
    tri = (np.arange(P)[None, :] - ii >= 0).astype(np.float32)  # [128,128]

    selones = np.zeros((P, 4 * P), dtype=np.float32)
    for hh in range(4):
        selones[:, hh * P + hh] = 1.0
    cpack = np.ascontiguousarray(
        np.concatenate([cos2, sin2, tri, selones], axis=1)).astype(NPBF)

    sel4 = np.zeros((4, 4 * P), dtype=NPBF)
    for hh in range(4):
        sel4[hh, hh * P:(hh + 1) * P] = 1.0
    return cpack, sel4


_CACHE = {}


def _get_built():
    if "nc" not in _CACHE:
        _CACHE["nc"] = build_bass()
        _CACHE["consts"] = _host_consts()
    return _CACHE["nc"], _CACHE["consts"]


def _swz(a, n_kt):
    """[n_kt*128, W] -> [128, n_kt, W] (partition-major swizzle), bf16."""
    w = a.shape[1]
    return np.ascontiguousarray(
        a.reshape(n_kt, P, w).transpose(1, 0, 2)).astype(NPBF)


def make_in_maps(x, Wq, bq, Wdown, Wk_up, Wv_up, Wo):
    cpack, sel4 = _get_built()[1]
    in_maps = []
    for c in range(N_CORES):
        b, hg = c // 2, c % 2
        sl = slice(hg * HL, (hg + 1) * HL)
        in_maps.append({
            "xT": _swz(np.ascontiguousarray(x[b].T), KT_H),
            "wq": _swz(Wq[:, sl], KT_H),
            "wdown": _swz(Wdown, KT_H),
            "wkup": _swz(Wk_up[:, sl], KT_L),
            "wvup": _swz(Wv_up[:, sl], KT_L),
            "wo": _swz(Wo[sl, :], NHL),
            "bq": np.ascontiguousarray(
                bq[sl].reshape(NHL, P).T).astype(np.float32),
            "cpack": cpack,
            "sel4": sel4,
        })
    return in_maps


def gather_out(results, bo):
    out = np.empty((B, S, HID), dtype=np.float32)
    for b in range(B):
        acc = results[2 * b]["outT"] + results[2 * b + 1]["outT"]  # [HID, S]
        out[b] = acc.T + bo[None, :]
    return out


def kernel(x, Wq, bq, Wdown, Wk_up, Wv_up, Wo, bo):
    x = np.asarray(x, dtype=np.float32)
    Wq = np.asarray(Wq, dtype=np.float32)
    bq = np.asarray(bq, dtype=np.float32)
    Wdown = np.asarray(Wdown, dtype=np.float32)
    Wk_up = np.asarray(Wk_up, dtype=np.float32)
    Wv_up = np.asarray(Wv_up, dtype=np.float32)
    Wo = np.asarray(Wo, dtype=np.float32)
    bo = np.asarray(bo, dtype=np.float32)

    nc, _ = _get_built()
    in_maps = make_in_maps(x, Wq, bq, Wdown, Wk_up, Wv_up, Wo)
    res = run_bass_kernel_spmd(nc, in_maps, core_ids=list(range(N_CORES)))
    return gather_out(res.results, bo)


# revision 25
# speedup vs baseline: 1.6929x; 1.0714x over previous
"""Trainium2 Bass kernel: MultiHeadLatentAttention (bf16 pipeline).

Problem (hardcoded): B=4, S=1024, HID=2048, NH=16 heads of HD=128, LAT=512,
fp32 in/out, causal attention with RoPE, latent-compressed K/V (MLA).

Sharding over 8 NeuronCores: core c = (batch b = c//2, head-group hg = c%2).
Each core handles one batch element and 8 heads (local width HL=1024).

All matmul operands are bf16 (host casts); PSUM accumulation is fp32
(bf16-everywhere measures ~5e-3 max-rel vs the 2e-2 gate).

Device layout (contraction dim always on SBUF partitions; all SBUF tiles
flat 2D [128, cols]):
  xT   [P, 16*S] bf16 (host pre-swizzled x[b].T), 4 batched DMAs
  QT = (x Wq + bq).T -> qT [P, 8*S];  latT = (x Wdown).T -> [P, 4*S]
  KT = (lat Wk_up).T -> kT [P, 8*S];  V natural -> v [P, 8*HL]
  RoPE per head-pair on [P, 2S] tiles: out = raw*cos2 + shift64(raw)*sin2e;
    the partition shift is two SBUF->SBUF DMAs issued from the SCALAR queue
    (HWDGE) so they never head-of-line-block the weight stream on Sync.
  scoresT_h = k_h @ q_h.T in [k,q] blocks; diagonal blocks column-sliced to
    widths 512/384/256/128, residual triangle zeroed by a tri mask.
  ex = exp(scores/sqrt(128)) bf16
  sums: per half-group of 4 heads one PSUM tile [4,512] accumulates
    sel-ones matmuls -> one reciprocal serves 4 heads.
  ctxT unnormalized bf16; normalized via bc = sel4^T @ rec broadcast matmul.
  out-proj of q-chunk 0 interleaved into attention of q-chunk 1.

DMA issue budget: weights/x batched into ~1MB transfers on Sync; rope
shifts + half the outT stores on Scalar (second HWDGE queue).

Host gathers: out[b] = (outT[2b] + outT[2b+1]).T + bo.
"""

import os

if "axon" not in os.environ.get("JAX_PLATFORMS", ""):
    os.environ["JAX_PLATFORMS"] = "axon"

import contextlib

import ml_dtypes
import numpy as np

import concourse.bacc as bacc
import concourse.mybir as mybir
import concourse.tile as tile
from concourse.bass_utils import run_bass_kernel_spmd

# ---- problem dims (hardcoded per contest rules)
B, S, HID, NH, LAT = 4, 1024, 2048, 16, 512
HD = 128
NHL = NH // 2          # heads per core = 8
HL = NHL * HD          # local head width = 1024
P = 128
KT_H = HID // P        # 16
KT_L = LAT // P        # 4
QCW = 512              # q-chunk width (PSUM bank = 512 fp32)
NQC = S // QCW         # 2
SC_SCALE = float(1.0 / np.sqrt(HD))

F32 = mybir.dt.float32
BF16 = mybir.dt.bfloat16
NPBF = ml_dtypes.bfloat16

N_CORES = 8
CPACK_W = 2 * S + 2 * S + P + 4 * P   # cos2 | sin2 | tri | selones128


def build_bass(loop_iters=None):
    nc = bacc.Bacc("TRN2", target_bir_lowering=False, debug=False, num_devices=8)

    xTd = nc.dram_tensor("xT", [P, KT_H, S], BF16, kind="ExternalInput")[:]
    wqd = nc.dram_tensor("wq", [P, KT_H, HL], BF16, kind="ExternalInput")[:]
    wdownd = nc.dram_tensor("wdown", [P, KT_H, LAT], BF16, kind="ExternalInput")[:]
    wkupd = nc.dram_tensor("wkup", [P, KT_L, HL], BF16, kind="ExternalInput")[:]
    wvupd = nc.dram_tensor("wvup", [P, KT_L, HL], BF16, kind="ExternalInput")[:]
    wod = nc.dram_tensor("wo", [P, NHL, HID], BF16, kind="ExternalInput")[:]
    bqd = nc.dram_tensor("bq", [P, NHL], F32, kind="ExternalInput")[:]
    cpackd = nc.dram_tensor("cpack", [P, CPACK_W], BF16, kind="ExternalInput")[:]
    sel4d = nc.dram_tensor("sel4", [4, 4 * P], BF16, kind="ExternalInput")[:]
    outTd = nc.dram_tensor("outT", [HID, S], F32, kind="ExternalOutput")[:]

    with tile.TileContext(nc) as tc, contextlib.ExitStack() as _les:
        if loop_iters is not None:
            _les.enter_context(tc.For_i(0, loop_iters, 1))
        with (
            tc.tile_pool(name="consts", bufs=1) as consts,
            tc.tile_pool(name="resident", bufs=1) as resident,
        ):
            cpack = consts.tile([P, CPACK_W], BF16)
            cos2_sb = cpack[:, 0:2 * S]
            sin2_sb = cpack[:, 2 * S:4 * S]
            tri_sb = cpack[:, 4 * S:4 * S + P]
            selo_sb = cpack[:, 4 * S + P:4 * S + P + 4 * P]
            bq_sb = consts.tile([P, NHL], F32)
            sel4_sb = consts.tile([4, 4 * P], BF16)

            latT = resident.tile([P, KT_L * S], BF16)
            qT = resident.tile([P, NHL * S], BF16)
            kT = resident.tile([P, NHL * S], BF16)
            v_sb = resident.tile([P, NHL * HL], BF16)
            ctxT = resident.tile([P, NHL * S], BF16)
            # phase-B weights, loaded during phase A (wvg also feeds the
            # V hl-half-1 filler inside phase C)
            wkg = resident.tile([P, KT_L * HL], BF16)
            wvg = resident.tile([P, KT_L * HL], BF16)

            pacc_cm = tc.tile_pool(name="pacc", bufs=8, space="PSUM")
            pacc = pacc_cm.__enter__()

            def rope_pair(rp, h, ps4, dst, bias, sin_eng,
                          add_eng=None, dma_eng=None):
                """RoPE for heads h, h+1 from 4 psum tiles [(j,ntc)]."""
                add_eng = add_eng or nc.vector
                dma_eng = dma_eng or nc.scalar
                raw = rp.tile([P, 2 * S], BF16, tag="raw", name="raw")
                sh = rp.tile([P, 2 * S], BF16, tag="sh", name="sh")
                for j in range(2):
                    for ntc in range(NQC):
                        seg = raw[:, (j * NQC + ntc) * QCW:
                                  (j * NQC + ntc + 1) * QCW]
                        if bias:
                            nc.scalar.add(seg, ps4[j * 2 + ntc],
                                          bq_sb[:, h + j:h + j + 1])
                        else:
                            nc.scalar.copy(seg, ps4[j * 2 + ntc])
                    # per-head shift: unblocks as soon as this head's two
                    # segment copies land (not the whole pair)
                    dma_eng.dma_start(sh[0:64, j * S:(j + 1) * S],
                                      raw[64:128, j * S:(j + 1) * S])
                    dma_eng.dma_start(sh[64:128, j * S:(j + 1) * S],
                                      raw[0:64, j * S:(j + 1) * S])
                out = dst[:, h * S:(h + 2) * S]
                nc.vector.tensor_mul(out, raw, cos2_sb)
                sin_eng.tensor_mul(sh, sh, sin2_sb)
                add_eng.tensor_add(out, out, sh)

            # ---------- phase A: QT (2 groups of 4 heads) + latT ----------
            with (
                tc.tile_pool(name="xp", bufs=1) as xp,
                tc.tile_pool(name="ws1", bufs=2) as ws1,
                tc.tile_pool(name="ropeA", bufs=2) as rpA,
            ):
                xT_sb = xp.tile([P, KT_H * S], BF16)
                # ramp-in: first x chunk + first weight chunk land ASAP so
                # the PE starts ~4us in, then the bulk streams behind them
                wg0 = ws1.tile([P, KT_H * QCW], BF16, tag="w", name="wg")
                nc.sync.dma_start(xT_sb[:, 0:2 * S], xTd[:, 0:2, :])
                nc.sync.dma_start(wg0[:, 0:2 * QCW], wqd[:, 0:2, 0:QCW])
                nc.sync.dma_start(xT_sb[:, 2 * S:8 * S], xTd[:, 2:8, :])
                nc.sync.dma_start(wg0[:, 2 * QCW:8 * QCW],
                                  wqd[:, 2:8, 0:QCW])
                nc.sync.dma_start(xT_sb[:, 8 * S:16 * S], xTd[:, 8:16, :])
                nc.sync.dma_start(wg0[:, 8 * QCW:16 * QCW],
                                  wqd[:, 8:16, 0:QCW])
                nc.sync.dma_start(bq_sb, bqd)
                # preload the exp table set while the PE is busy with
                # projections (first ACTIVATE otherwise pays ~2.7us in C)
                warm = ws1.tile([1, NHL], F32, tag="warm", name="warm")
                nc.scalar.activation(
                    out=warm, in_=bq_sb[0:1, :],
                    func=mybir.ActivationFunctionType.Exp, scale=1.0)

                for og in range(2):
                    if og == 0:
                        wg = wg0
                    else:
                        wg = ws1.tile([P, KT_H * QCW], BF16, tag="w",
                                      name="wg")
                        for hf in range(2):   # two 1MB halves
                            nc.sync.dma_start(
                                wg[:, hf * 8 * QCW:(hf + 1) * 8 * QCW],
                                wqd[:, hf * 8:(hf + 1) * 8,
                                    og * QCW:(og + 1) * QCW])
                    ps = [pacc.tile([P, QCW], F32, tag="acc", name="acc")
                          for _ in range(8)]
                    for kt in range(KT_H):
                        for oi in range(4):
                            for ntc in range(NQC):
                                nc.tensor.matmul(
                                    ps[oi * 2 + ntc],
                                    lhsT=wg[:, kt * QCW + oi * P:
                                            kt * QCW + (oi + 1) * P],
                                    rhs=xT_sb[:, kt * S + ntc * QCW:
                                              kt * S + (ntc + 1) * QCW],
                                    start=(kt == 0),
                                    stop=(kt == KT_H - 1),
                                )
                    if og == 0:
                        nc.sync.dma_start(cpack, cpackd)
                        nc.sync.dma_start(sel4_sb, sel4d)
                    for pr in range(2):
                        rope_pair(rpA, og * 4 + pr * 2,
                                  ps[pr * 4:pr * 4 + 4], qT, bias=True,
                                  sin_eng=nc.gpsimd)

                # latT group (4 out tiles x 2 chunks)
                wg = ws1.tile([P, KT_H * QCW], BF16, tag="w", name="wg")
                for hf in range(2):
                    nc.sync.dma_start(
                        wg[:, hf * 8 * QCW:(hf + 1) * 8 * QCW],
                        wdownd[:, hf * 8:(hf + 1) * 8, :])
                nc.sync.dma_start(wkg, wkupd)
                nc.sync.dma_start(wvg, wvupd)
                ps = [pacc.tile([P, QCW], F32, tag="acc", name="acc")
                      for _ in range(8)]
                for kt in range(KT_H):
                    for oi in range(4):
                        for ntc in range(NQC):
                            nc.tensor.matmul(
                                ps[oi * 2 + ntc],
                                lhsT=wg[:, kt * QCW + oi * P:
                                        kt * QCW + (oi + 1) * P],
                                rhs=xT_sb[:, kt * S + ntc * QCW:
                                          kt * S + (ntc + 1) * QCW],
                                start=(kt == 0),
                                stop=(kt == KT_H - 1),
                            )
                for oi in range(4):
                    for ntc in range(NQC):
                        dstap = latT[:, oi * S + ntc * QCW:
                                     oi * S + (ntc + 1) * QCW]
                        if ntc == 0:
                            nc.scalar.copy(dstap, ps[oi * 2 + ntc])
                        else:
                            nc.vector.tensor_copy(dstap, ps[oi * 2 + ntc])

            # ---------- phase B: KT (rope) then V hl-half 0 ----------
            # (wkup/wvup were loaded during phase A; wo loads during B;
            #  V hl-half 1 is emitted later as PE filler inside qc0
            #  attention, using the pctx pool.)
            wop_cm = tc.tile_pool(name="wop", bufs=1)
            wop = wop_cm.__enter__()
            wo_sb = wop.tile([P, NHL * HID], BF16)
            nc.sync.dma_start(wo_sb, wod)

            rpB_cm = tc.tile_pool(name="ropeB", bufs=2)
            rpB = rpB_cm.__enter__()

            for og in range(2):
                ps = [pacc.tile([P, QCW], F32, tag="acc", name="acc")
                      for _ in range(8)]
                for kt in range(KT_L):
                    for oi in range(4):
                        for ntc in range(NQC):
                            nc.tensor.matmul(
                                ps[oi * 2 + ntc],
                                lhsT=wkg[:, kt * HL + og * 4 * P + oi * P:
                                         kt * HL + og * 4 * P
                                         + (oi + 1) * P],
                                rhs=latT[:, kt * S + ntc * QCW:
                                         kt * S + (ntc + 1) * QCW],
                                start=(kt == 0),
                                stop=(kt == KT_L - 1),
                            )
                for pr in range(2):
                    sin = nc.vector if og == 0 else nc.gpsimd
                    rope_pair(rpB, og * 4 + pr * 2,
                              ps[pr * 4:pr * 4 + 4], kT, bias=False,
                              sin_eng=sin, add_eng=nc.vector,
                              dma_eng=nc.sync)

            for hlc in range(2):
                ps = [pacc.tile([P, QCW], F32, tag="acc", name="acc")
                      for _ in range(8)]
                for kt in range(KT_L):
                    for st in range(8):
                        nc.tensor.matmul(
                            ps[st],
                            lhsT=latT[:, kt * S + st * P:
                                      kt * S + (st + 1) * P],
                            rhs=wvg[:, kt * HL + hlc * QCW:
                                    kt * HL + (hlc + 1) * QCW],
                            start=(kt == 0),
                            stop=(kt == KT_L - 1),
                        )
                for st in range(8):
                    dstap = v_sb[:, st * HL + hlc * QCW:
                                 st * HL + (hlc + 1) * QCW]
                    if st % 4 == 1:
                        nc.vector.tensor_copy(dstap, ps[st])
                    else:
                        nc.scalar.copy(dstap, ps[st])

            rpB_cm.__exit__(None, None, None)
            pacc_cm.__exit__(None, None, None)

            # ---------- phase C: attention + out-projection ----------
            with (
                tc.tile_pool(name="psc", bufs=2, space="PSUM") as psc,
                tc.tile_pool(name="pctx", bufs=2, space="PSUM") as pctx,
                tc.tile_pool(name="psums", bufs=2, space="PSUM") as psums,
                tc.tile_pool(name="pbc", bufs=2, space="PSUM") as pbc,
                tc.tile_pool(name="exla", bufs=3) as exla,
                tc.tile_pool(name="exlb", bufs=3) as exlb,
                tc.tile_pool(name="small", bufs=2) as small,
                tc.tile_pool(name="outsb", bufs=3) as outsb,
            ):
                # ---- fine-grained PE filler: each closure emits ~one
                # always-ready matmul (out-proj accumulation step or a
                # V hl-half-1 accumulation step), popped between attention
                # dependency steps to keep the PE dense and warm.
                fill = []

                def fl_pop(n):
                    for _ in range(n):
                        if fill:
                            fill.pop(0)()

                def add_outproj(qc, ot):
                    st_ = {}

                    def mk_mm(kt):
                        def go():
                            if kt == 0:
                                st_["po"] = pbc.tile([P, QCW], F32,
                                                     tag="bcpo", name="po")
                            nc.tensor.matmul(
                                st_["po"],
                                lhsT=wo_sb[:, kt * HID + ot * P:
                                           kt * HID + (ot + 1) * P],
                                rhs=ctxT[:, kt * S + qc * QCW:
                                         kt * S + (qc + 1) * QCW],
                                start=(kt == 0),
                                stop=(kt == NHL - 1),
                            )
                        return go

                    def fin():
                        ob = outsb.tile([P, QCW], F32, tag="osb", name="ob")
                        nc.vector.tensor_copy(ob, st_["po"])
                        nc.sync.dma_start(
                            outTd[ot * P:(ot + 1) * P,
                                  qc * QCW:(qc + 1) * QCW], ob)

                    for kt in range(NHL):
                        fill.append(mk_mm(kt))
                    fill.append(fin)


                def att_unit(h, hh, qc, sums_ps):
                    """Generator: one attention head, yields per kt step."""
                    nkt = 4 * qc + 4
                    ctx = pctx.tile([P, QCW], F32, tag="ctx", name="ctx")

                    def geom(kt):
                        off = kt - 4 * qc
                        if off < 0:
                            return 0, QCW, False
                        return 128 * off, QCW - 128 * off, True

                    def emit_sc(kt):
                        lo, w, diag = geom(kt)
                        sc = psc.tile([P, QCW], F32, tag="sc", name="sc")
                        nc.tensor.matmul(
                            sc[:, :w],
                            lhsT=kT[:, h * S + kt * P:h * S + (kt + 1) * P],
                            rhs=qT[:, h * S + qc * QCW + lo:
                                   h * S + qc * QCW + lo + w],
                            start=True, stop=True,
                        )
                        exp_pool = exla if kt % 2 == 0 else exlb
                        ex = exp_pool.tile([P, QCW], BF16, tag="ex",
                                           name="ex")
                        nc.scalar.activation(
                            out=ex[:, :w], in_=sc[:, :w],
                            func=mybir.ActivationFunctionType.Exp,
                            scale=SC_SCALE,
                        )
                        if diag:
                            # gpsimd only late in C (it chews kT h4-7 rope
                            # early on)
                            eng = (nc.gpsimd if (qc == 1 and h >= 4
                                                 and kt % 2 == 0)
                                   else nc.vector)
                            eng.tensor_mul(ex[:, 0:P], ex[:, 0:P], tri_sb)
                        return ex

                    def emit_pv(kt, ex):
                        lo, w, _ = geom(kt)
                        nc.tensor.matmul(
                            ctx[:, lo:lo + w],
                            lhsT=v_sb[:, kt * HL + h * P:
                                      kt * HL + (h + 1) * P],
                            rhs=ex[:, :w],
                            start=(kt == 0),
                            stop=(kt == nkt - 1),
                        )
                        nc.tensor.matmul(
                            sums_ps[:, lo:lo + w],
                            lhsT=selo_sb[:, hh * P:(hh + 1) * P],
                            rhs=ex[:, :w],
                            start=(hh == 0 and kt == 0),
                            stop=(hh == 3 and kt == nkt - 1),
                        )

                    exs = {0: emit_sc(0)}
                    for kt in range(nkt):
                        if kt + 1 < nkt:
                            exs[kt + 1] = emit_sc(kt + 1)
                        emit_pv(kt, exs.pop(kt))
                        yield
                    ctx_dst = ctxT[:, h * S + qc * QCW:
                                   h * S + (qc + 1) * QCW]
                    if qc == 0:
                        nc.scalar.copy(ctx_dst, ctx)
                    else:
                        nc.vector.tensor_copy(ctx_dst, ctx)

                def drive(gens, on_done=None):
                    live = list(gens)
                    while live:
                        nxt = []
                        for g in live:
                            try:
                                next(g)
                                nxt.append(g)
                            except StopIteration:
                                if on_done is not None:
                                    on_done(g)
                            fl_pop(1)
                        live = nxt

                pending_norm = []

                def flush_norm():
                    while pending_norm:
                        pending_norm.pop(0)()

                def finish_group(qc, half, sums_ps):
                    # copy the sums rows out to free the PSUM bank fast,
                    # then reciprocal off the SBUF copy
                    srow = small.tile([4, QCW], F32, tag="srow",
                                      name="srow")
                    nc.vector.tensor_copy(srow, sums_ps[0:4, :])
                    rec = small.tile([4, QCW], BF16, tag="rec", name="rec")
                    with nc.allow_low_precision(reason="bf16 softmax "
                                                "denominator (gate 2e-2)"):
                        nc.vector.reciprocal(out=rec, in_=srow)

                    def go():
                        for hh in range(4):
                            h = half * 4 + hh
                            bc = pbc.tile([P, QCW], F32, tag="bcpo",
                                          name="bc")
                            nc.tensor.matmul(
                                bc,
                                lhsT=sel4_sb[:, hh * P:(hh + 1) * P],
                                rhs=rec,
                                start=True, stop=True,
                            )
                            sl = ctxT[:, h * S + qc * QCW:
                                      h * S + (qc + 1) * QCW]
                            nc.vector.tensor_mul(sl, sl, bc)
                            fl_pop(2)
                    return go

                # two passes: (qc0 h || qc1 h) paired per head — the short
                # qc0 stream and long qc1 stream hide each other's
                # dependency stalls on the in-order PE queue
                for half in range(2):
                    sums_a = psums.tile([P, QCW], F32, tag="sums",
                                        name="sums")
                    sums_b = psums.tile([P, QCW], F32, tag="sums",
                                        name="sums")
                    for hh in range(4):
                        h = half * 4 + hh
                        drive([att_unit(h, hh, 0, sums_a),
                               att_unit(h, hh, 1, sums_b)])
                        if hh == 0:
                            flush_norm()
                    norm_a = finish_group(0, half, sums_a)
                    norm_b = finish_group(1, half, sums_b)
                    if half == 0:
                        pending_norm += [norm_a, norm_b]
                    else:
                        norm_a()
                        for ot in range(HID // P):
                            add_outproj(0, ot)
                        norm_b()
                        for ot in range(HID // P):
                            add_outproj(1, ot)
                while fill:
                    fill.pop(0)()

            wop_cm.__exit__(None, None, None)
    nc.compile()
    return nc


# ---------------- host side ----------------

def _host_consts():
    inv_freq = 1.0 / (10000.0 ** (np.arange(0, HD, 2, dtype=np.float64) / HD))
    t = np.arange(S, dtype=np.float64)
    freqs = t[:, None] * inv_freq[None, :]            # [S, 64]
    emb = np.concatenate([freqs, freqs], axis=-1)     # [S, 128]
    cosT = np.cos(emb).T.astype(np.float32)           # [128, S]
    sinT = np.sin(emb).T.astype(np.float32)
    sinTe = sinT.copy()
    sinTe[:64] *= -1.0                                # rotate_half sign folded
    cos2 = np.broadcast_to(cosT[:, None, :], (P, 2, S)).reshape(P, 2 * S)
    sin2 = np.broadcast_to(sinTe[:, None, :], (P, 2, S)).reshape(P, 2 * S)

    ii = np.arange(P)[:, None]
    tri = (np.arange(P)[None, :] - ii >= 0).astype(np.float32)  # [128,128]

    selones = np.zeros((P, 4 * P), dtype=np.float32)
    for hh in range(4):
        selones[:, hh * P + hh] = 1.0
    cpack = np.ascontiguousarray(
        np.concatenate([cos2, sin2, tri, selones], axis=1)).astype(NPBF)

    sel4 = np.zeros((4, 4 * P), dtype=NPBF)
    for hh in range(4):
        sel4[hh, hh * P:(hh + 1) * P] = 1.0
    return cpack, sel4


_CACHE = {}


def _get_built():
    if "nc" not in _CACHE:
        _CACHE["nc"] = build_bass()
        _CACHE["consts"] = _host_consts()
    return _CACHE["nc"], _CACHE["consts"]


def _swz(a, n_kt):
    """[n_kt*128, W] -> [128, n_kt, W] (partition-major swizzle), bf16."""
    w = a.shape[1]
    return np.ascontiguousarray(
        a.reshape(n_kt, P, w).transpose(1, 0, 2)).astype(NPBF)


def make_in_maps(x, Wq, bq, Wdown, Wk_up, Wv_up, Wo):
    cpack, sel4 = _get_built()[1]
    in_maps = []
    for c in range(N_CORES):
        b, hg = c // 2, c % 2
        sl = slice(hg * HL, (hg + 1) * HL)
        in_maps.append({
            "xT": _swz(np.ascontiguousarray(x[b].T), KT_H),
            "wq": _swz(Wq[:, sl], KT_H),
            "wdown": _swz(Wdown, KT_H),
            "wkup": _swz(Wk_up[:, sl], KT_L),
            "wvup": _swz(Wv_up[:, sl], KT_L),
            "wo": _swz(Wo[sl, :], NHL),
            "bq": np.ascontiguousarray(
                bq[sl].reshape(NHL, P).T).astype(np.float32),
            "cpack": cpack,
            "sel4": sel4,
        })
    return in_maps


def gather_out(results, bo):
    out = np.empty((B, S, HID), dtype=np.float32)
    for b in range(B):
        acc = results[2 * b]["outT"] + results[2 * b + 1]["outT"]  # [HID, S]
        out[b] = acc.T + bo[None, :]
    return out


def kernel(x, Wq, bq, Wdown, Wk_up, Wv_up, Wo, bo):
    x = np.asarray(x, dtype=np.float32)
    Wq = np.asarray(Wq, dtype=np.float32)
    bq = np.asarray(bq, dtype=np.float32)
    Wdown = np.asarray(Wdown, dtype=np.float32)
    Wk_up = np.asarray(Wk_up, dtype=np.float32)
    Wv_up = np.asarray(Wv_up, dtype=np.float32)
    Wo = np.asarray(Wo, dtype=np.float32)
    bo = np.asarray(bo, dtype=np.float32)

    nc, _ = _get_built()
    in_maps = make_in_maps(x, Wq, bq, Wdown, Wk_up, Wv_up, Wo)
    res = run_bass_kernel_spmd(nc, in_maps, core_ids=list(range(N_CORES)))
    return gather_out(res.results, bo)
